# revision 63
# baseline (speedup 1.0000x reference)
"""GCN layer Trainium2 Bass kernel, 8-core SPMD. v4.

Math: out = norm * (segment_sum_dst(gather_src(norm * (h @ W)))) + b
Rewritten (matmul commutes with the linear aggregation):
    out = norm_dst * ((A @ (norm_src * h)) @ W) + b

v3: host pre-scales hn = (norm * h) and casts to bf16, so the device
gathers 256B bf16 rows straight into the PE-ready message buffer; gather
calls round-robin over 4 SWDGE queues.

v4 (this version), guided by the CoreSim cost model (the kernel is bound
by the 16-DMA-engine gather stream, ~93.5us/core; everything else must
hide under it):
  - split gidx load (first 2 superwindows land first) so the gather
    stream starts ~1us in; small loads ride between the two parts.
  - per-(slot,queue) gather semaphores (one SWDGE queue per semaphore,
    required by the ucode/CoreSim) + per-HALF gather waits on PE, which
    trims one call's drain latency off the tail.
  - bias folded into PSUM via a rank-1 ones x bias matmul; the
    post-aggregation scale moved from DVE (scalar_tensor_tensor) to ACT
    (Copy with per-partition scale), freeing DVE for the one-hot builds.
  - aggTs/W in bf16: window matmuls run 1 cycle/row instead of fp32's 4.
  - paired-row bf16 output (partition p holds dst rows 2p,2p+1): 64
    512B output descriptors per window, half the write bytes; out tensor
    is [NW*64, 2F] bf16, upcast + reshaped on host.
  - per-superwindow swept src-table split points: the int16 index limit
    allows any split in [N-32768, 32768], and each sw's gather calls use
    their own table-slice offset; host_prep sweeps each to minimize
    padded tiles (~5% fewer descriptors than 25000/25000). Edges are
    src-sorted within each gather call for HBM locality.
"""
import os
import sys
import numpy as np

for _p in ("/opt/trn_rl_repo",):
    if _p not in sys.path and os.path.isdir(_p):
        sys.path.insert(0, _p)

import ml_dtypes

import concourse.bacc as bacc
import concourse.bass as bass
import concourse.mybir as mybir
from concourse.alu_op_type import AluOpType
from concourse.library_config import mlp as mlp_library

BF16 = ml_dtypes.bfloat16

# ---------------- problem constants (hardcoded per contract) ----------------
N, F, E, C = 50000, 128, 800000, 8
R = N // C                      # 6250 dst rows per core
WIN = 128                       # segs per window (PSUM partition dim)
NW = (R + WIN - 1) // WIN       # 49 windows per core
HALF = 25000                    # fallback src split (host_prep sweeps the real one)
SW = 2                          # windows per superwindow (gather-call granularity)
TILE = 128                      # edges per tile (PE contraction dim)
GCHUNK = 8                      # max tiles (x128 descriptors) per dma_gather call
NQ = 4                          # SWDGE queues (round-robin per gather call)
DMA_SCRATCH = 16384             # SWDGE ring: 16384/16 = 1024 desc per queue
                                # (>=2304-desc calls wedge the HW ucode; 1024 is the
                                # empirically safe per-call cap from the v3 session)


# ---------------------------- host preprocessing ----------------------------

def host_prep(h, norm, W, b, src, dst):
    src = np.ascontiguousarray(np.asarray(src).astype(np.int64))
    dst = np.ascontiguousarray(np.asarray(dst).astype(np.int64))
    norm_f = np.asarray(norm, dtype=np.float32).reshape(-1)

    core = dst // R
    w_of = (dst % R) // WIN

    # src-table split point for int16 gather indices: any P with
    # N-32768 <= P <= 32768 is addressable (lo idx = src < P, hi idx =
    # src - P). Each superwindow's gather calls may use their OWN split
    # (the table slice is per call), so sweep P per sw to minimize that
    # sw's padded tiles: asymmetric splits waste less than 25000/25000.
    n_sw_t = (NW + SW - 1) // SW
    base_key = (core * NW + w_of) * 2
    Pcands = list(range(max(N - 32768 + 48, 17280), 32769, 128))
    tiles_mat = np.zeros((len(Pcands), n_sw_t), dtype=np.int64)
    for i, Pc in enumerate(Pcands):
        k = base_key + (src >= Pc)
        sz = np.bincount(k, minlength=C * NW * 2).reshape(C, NW, 2)
        per_w = (np.maximum(1, -(-sz[:, :, 0].max(axis=0) // TILE))
                 + np.maximum(1, -(-sz[:, :, 1].max(axis=0) // TILE)))
        for s in range(n_sw_t):
            tiles_mat[i, s] = per_w[s * SW:(s + 1) * SW].sum()
    Ps = [int(Pcands[int(np.argmin(tiles_mat[:, s]))]) for s in range(n_sw_t)]
    sw_of_edge = w_of // SW
    half = (src >= np.asarray(Ps)[sw_of_edge]).astype(np.int64)

    key = (core * NW + w_of) * 2 + half
    # secondary sort by src: ascending gather addresses within each call
    # (HBM row-buffer locality for the descriptor stream)
    order = np.lexsort((src, key))
    src_s, dst_s = src[order], dst[order]
    key_s = key[order]

    sizes = np.bincount(key_s, minlength=C * NW * 2).reshape(C, NW, 2)
    starts = np.zeros(C * NW * 2 + 1, dtype=np.int64)
    np.cumsum(sizes.reshape(-1), out=starts[1:])

    # tiles per (window, half): max over cores, both halves forced >= 1
    Tlo = np.maximum(1, -(-sizes[:, :, 0].max(axis=0) // TILE))
    Thi = np.maximum(1, -(-sizes[:, :, 1].max(axis=0) // TILE))

    n_sw = (NW + SW - 1) // SW
    sw_windows = [list(range(s * SW, min((s + 1) * SW, NW))) for s in range(n_sw)]

    # global tile list: per sw, big half's groups first, small half last
    # (fewer tiles depend on the stream's final drain -> shorter tail)
    hi_frac = float(np.mean(half))
    half_order = (1, 0) if hi_frac >= 0.5 else (0, 1)
    tile_window, tile_half = [], []
    call_list = []  # (sw, half, tile_start, n_tiles_call)
    for s in range(n_sw):
        for hf in half_order:
            t0 = len(tile_window)
            for w in sw_windows[s]:
                T = int((Tlo if hf == 0 else Thi)[w])
                tile_window += [w] * T
                tile_half += [hf] * T
            call_list.append((s, hf, t0, len(tile_window) - t0))
    tile_window = np.asarray(tile_window)
    tile_half = np.asarray(tile_half)
    n_tiles = len(tile_window)

    # per-window first/last tile (for matmul start/stop flags)
    first_tile = {}
    last_tile = {}
    for t, w in enumerate(tile_window):
        w = int(w)
        first_tile.setdefault(w, t)
        last_tile[w] = t

    # tiles per sw (buffer sizing) & tile offset within sw
    sw_tile_start = {}
    tile_off_in_sw = np.zeros(n_tiles, dtype=np.int64)
    for s, hf, t0, ntc in call_list:
        sw_tile_start.setdefault(s, t0)
    maxT_sw = 0
    for s in range(n_sw):
        t0 = sw_tile_start[s]
        t1 = sw_tile_start[s + 1] if s + 1 in sw_tile_start else n_tiles
        maxT_sw = max(maxT_sw, t1 - t0)
        tile_off_in_sw[t0:t1] = np.arange(t1 - t0)

    pattern = dict(
        Ps=Ps,
        Tlo=Tlo, Thi=Thi, n_tiles=n_tiles, n_sw=n_sw, sw_windows=sw_windows,
        tile_window=tile_window, tile_half=tile_half, call_list=call_list,
        first_tile=first_tile, last_tile=last_tile, maxT_sw=maxT_sw,
        sw_tile_start=sw_tile_start, tile_off_in_sw=tile_off_in_sw,
    )

    # ---- per-core data ----
    cores = []
    W_np = np.asarray(W, dtype=np.float32).astype(BF16)
    b_np = np.asarray(b, dtype=np.float32).reshape(-1)
    bias_row = b_np[None, :].astype(BF16)          # [1, F] rank-1 bias matmul rhs
    ones_row = np.ones((1, WIN), dtype=np.float32).astype(BF16)
    iota_bf = np.tile(np.arange(WIN, dtype=np.float32)[None, :], (128, 1)).astype(BF16)
    # pre-scaled bf16 features: the gathered payload needs no on-device scaling
    hn_np = np.ascontiguousarray(
        (np.asarray(h, dtype=np.float32) * norm_f[:, None]).astype(BF16))

    for c in range(C):
        gidx_flat = np.zeros(n_tiles * TILE, dtype=np.int16)
        # pad slots get dst -1: their one-hot S row is all-zero, so the
        # garbage rows the pad descriptors gather never contribute
        mdst_flat = np.full(n_tiles * TILE, -1.0, dtype=np.float32)
        for s, hf, t0, ntc in call_list:
            pos = t0 * TILE
            for w in sw_windows[s]:
                g = (c * NW + w) * 2 + hf
                st, en = int(starts[g]), int(starts[g + 1])
                n = en - st
                T = int((Tlo if hf == 0 else Thi)[w])
                sl = slice(pos, pos + n)
                gsrc = src_s[st:en]
                gidx_flat[sl] = (gsrc - hf * Ps[s]).astype(np.int16)
                mdst_flat[sl] = (dst_s[st:en] - c * R - w * WIN).astype(np.float32)
                pos += T * TILE

        # gather idx wrapped layout per call: [16, n/16] blocks, tiled x8
        blocks = []
        for s, hf, t0, ntc in call_list:
            fl = gidx_flat[t0 * TILE:(t0 + ntc) * TILE]
            blocks.append(fl.reshape(-1, 16).T)           # [16, ntc*8]
        gidx_wrapped = np.tile(np.concatenate(blocks, axis=1), (8, 1))  # [128, n_tiles*8]

        mdst_t = mdst_flat.reshape(n_tiles, TILE).T.copy()  # [128, n_tiles]

        nd = np.zeros((WIN, NW), dtype=np.float32)
        for w in range(NW):
            lo = c * R + w * WIN
            hi = min(lo + WIN, (c + 1) * R)
            nd[: hi - lo, w] = norm_f[lo:hi]
        # paired-row output scales: partition p writes dst rows (2p, 2p+1)
        nd2 = np.zeros((WIN // 2, 2, NW), dtype=np.float32)
        nd2[:, 0, :] = nd[0::2, :]
        nd2[:, 1, :] = nd[1::2, :]

        cores.append({
            "hn": hn_np,
            "gidx": np.ascontiguousarray(gidx_wrapped),
            "mdst": np.ascontiguousarray(mdst_t),
            "norm_dst": np.ascontiguousarray(nd2),
            "Wmat": W_np,
            "bias_row": bias_row,
            "ones_row": ones_row,
            "iota_bf": iota_bf,
        })
    return cores, pattern


# ----------------------------- device program -------------------------------

def build_program(pat):
    Ps = pat["Ps"]
    n_tiles = pat["n_tiles"]
    n_sw = pat["n_sw"]
    maxT = pat["maxT_sw"]
    tile_window = pat["tile_window"]
    tile_half = pat["tile_half"]
    first_tile, last_tile = pat["first_tile"], pat["last_tile"]
    call_list = pat["call_list"]
    sw_windows = pat["sw_windows"]
    sw_tile_start = pat["sw_tile_start"]
    tile_off = pat["tile_off_in_sw"]

    def sw_tiles(s):
        t0 = sw_tile_start[s]
        t1 = sw_tile_start[s + 1] if s + 1 in sw_tile_start else n_tiles
        return list(range(t0, t1))

    # ---- schedules & counters (for cross-engine wait targets) ----
    # PE op order: per sw: tiles, then per previous-sw window: bias-mm + W-mm.
    pe_count_after = {}   # key: ("tile", t) / ("bmm", w) / ("wmm", w) -> pe_c after op
    cnt = 0
    for s in range(n_sw + 1):
        if s < n_sw:
            for t in sw_tiles(s):
                cnt += 1
                pe_count_after[("tile", t)] = cnt
        if s >= 1:
            for w in sw_windows[s - 1]:
                cnt += 2
                pe_count_after[("bmm", w)] = cnt
                cnt += 2
                pe_count_after[("wmm", w)] = cnt
    pe_after_tiles_of_sw = {}
    for s in range(n_sw):
        pe_after_tiles_of_sw[s] = pe_count_after[("tile", sw_tiles(s)[-1])]

    dt = mybir.dt
    nc = bacc.Bacc("TRN2", debug=False, num_swdge_queues=NQ,
                   dynamic_dma_scratch_size=DMA_SCRATCH)

    hn_d = nc.dram_tensor("hn", [N, F], dt.bfloat16, kind="ExternalInput")
    gidx_d = nc.dram_tensor("gidx", [128, n_tiles * 8], dt.int16, kind="ExternalInput")
    mdst_d = nc.dram_tensor("mdst", [128, n_tiles], dt.float32, kind="ExternalInput")
    nd_d = nc.dram_tensor("norm_dst", [WIN // 2, 2, NW], dt.float32, kind="ExternalInput")
    W_d = nc.dram_tensor("Wmat", [F, F], dt.bfloat16, kind="ExternalInput")
    bias_d = nc.dram_tensor("bias_row", [1, F], dt.bfloat16, kind="ExternalInput")
    ones_d = nc.dram_tensor("ones_row", [1, WIN], dt.bfloat16, kind="ExternalInput")
    iota_d = nc.dram_tensor("iota_bf", [128, WIN], dt.bfloat16, kind="ExternalInput")
    # paired-row output: partition p holds dst rows (2p, 2p+1) -> 1KB descriptors
    out_d = nc.dram_tensor("out", [NW * WIN // 2, 2 * F], dt.bfloat16,
                           kind="ExternalOutput")

    sb_gidx = nc.alloc_sbuf_tensor("sb_gidx", [128, n_tiles * 8], dt.int16)
    sb_mdst = nc.alloc_sbuf_tensor("sb_mdst", [128, n_tiles], dt.float32)
    sb_nd = nc.alloc_sbuf_tensor("sb_nd", [WIN // 2, 2, NW], dt.float32)
    sb_W = nc.alloc_sbuf_tensor("sb_W", [F, F], dt.bfloat16)
    sb_bias = nc.alloc_sbuf_tensor("sb_bias", [1, F], dt.bfloat16)
    sb_ones = nc.alloc_sbuf_tensor("sb_ones", [1, WIN], dt.bfloat16)
    sb_iota = nc.alloc_sbuf_tensor("sb_iota", [128, WIN], dt.bfloat16)

    mbuf = nc.alloc_sbuf_tensor("mbuf", [128, 2, maxT, TILE], dt.bfloat16)
    sbuf_S = nc.alloc_sbuf_tensor("sbuf_S", [128, 2, maxT, WIN], dt.bfloat16)
    aggTs = nc.alloc_sbuf_tensor("aggTs", [F, 2, WIN], dt.bfloat16)
    outsb = nc.alloc_sbuf_tensor("outsb", [WIN // 2, 2, 2 * F], dt.bfloat16)

    # PSUM: agg slots (w%4) in banks 0-3; paired ps_out in banks 4-7
    # (window w uses bank pair 2*(w%2)+{0,1}: even dst rows / odd dst rows)
    ps_agg = nc.alloc_psum_tensor("ps_agg", [128, 4, 512], dt.float32)
    ps_out = nc.alloc_psum_tensor("ps_out", [128, 4, 512], dt.float32)

    ld_g0 = nc.alloc_semaphore("ld_g0")      # gidx part 0 (first call)
    ld_g = nc.alloc_semaphore("ld_g")        # gidx part 1 (rest of first 2 sw)
    ld_g2 = nc.alloc_semaphore("ld_g2")      # gidx part 2 (rest)
    ld_dve = nc.alloc_semaphore("ld_dve")    # mdst + iota
    ld_rest = nc.alloc_semaphore("ld_rest")  # nd + ones + bias + W
    # per-(slot, half, queue) gather sems: a semaphore may only be updated from
    # one SWDGE queue, and consecutive updates of one sem must be separated by
    # an acknowledged sync point (the gp's mbuf-reuse wait on pe_c provides it
    # between same-parity superwindows)
    gsem = [[[nc.alloc_semaphore(f"gsem{sl}_{hf}_{q}") for q in range(NQ)]
             for hf in range(2)] for sl in range(2)]
    sready = nc.alloc_semaphore("sready")
    pe_c = nc.alloc_semaphore("pe_c")
    aggc = nc.alloc_semaphore("aggc")
    dvsc = nc.alloc_semaphore("dvsc")
    osem = [nc.alloc_semaphore("osem0"), nc.alloc_semaphore("osem1")]


    # Replay the chunking loop to assign queues and cumulative sem targets.
    # Each (s,hf) group's chunks use queues round-robin from a global counter;
    # a group must not hit one queue twice (one sem update per round).
    # chunk_target[t] gives the (sem-slot, hf, queue, value) wait that covers
    # the chunk containing tile t, so PE can consume chunk-by-chunk as each
    # call drains instead of waiting for the whole half.
    _cum = {(sl, hf, q): 0 for sl in range(2) for hf in range(2) for q in range(NQ)}
    chunk_target = {}  # first tile of chunk -> (s%2, hf, q, value)
    qrr = 0
    for s, hf, t0, ntc in call_list:
        grp_q = []
        for c0 in range(0, ntc, GCHUNK):
            q = qrr % NQ
            assert q not in grp_q, "chunk count per (sw,half) exceeds NQ"
            grp_q.append(q)
            _cum[(s % 2, hf, q)] += 16
            chunk_target[t0 + c0] = (s % 2, hf, q, _cum[(s % 2, hf, q)])
            qrr += 1

    with nc.Block() as block:

        @block.sync
        def _(sync: bass.BassEngine):
            # split load sems so each engine waits only on what it reads:
            # gathers need gidx; DVE S-build needs mdst+iota; ACT/PE window
            # work needs nd/ones/bias/W. gidx lands in three parts: the first
            # call's slice (tiny, unblocks the stream ~1us earlier), the rest
            # of the first two sws, then everything else.
            g_split = sw_tile_start[2] * 8 if n_sw > 2 else n_tiles * 8
            sync.dma_start(sb_gidx[:, 0:g_split],
                           gidx_d[:, 0:g_split]).then_inc(ld_g0, 16)
            sync.dma_start(sb_mdst[:, :], mdst_d[:, :]).then_inc(ld_dve, 16)
            sync.dma_start(sb_iota[:, :], iota_d[:, :]).then_inc(ld_dve, 16)
            sync.dma_start(sb_nd[:, :], nd_d[:, :]).then_inc(ld_rest, 16)
            sync.dma_start(sb_ones[:, :], ones_d[:, :]).then_inc(ld_rest, 16)
            sync.dma_start(sb_bias[:, :], bias_d[:, :]).then_inc(ld_rest, 16)
            sync.dma_start(sb_W[:, :], W_d[:, :]).then_inc(ld_rest, 16)
            if g_split < n_tiles * 8:
                # rest of gidx last: off the small loads' critical path
                sync.dma_start(sb_gidx[:, g_split:],
                               gidx_d[:, g_split:]).then_inc(ld_g2, 16)
            HW2 = WIN // 2
            for w in range(NW):
                sync.wait_ge(dvsc, w + 1)
                sync.dma_start(
                    out_d[w * HW2:(w + 1) * HW2, :], outsb[:, w % 2, :]
                ).then_inc(osem[w % 2], 16)

        @block.gpsimd
        def _(gp: bass.BassGpSimd):
            gp.load_library(mlp_library)
            gp.wait_ge(ld_g0, 16)  # first 2 sws' gidx landed
            qrr = 0
            waited_g2 = False
            for s, hf, t0, ntc in call_list:
                if s >= 2 and not waited_g2:
                    gp.wait_ge(ld_g2, 16)  # rest of sb_gidx landed
                    waited_g2 = True
                if t0 == sw_tile_start[s] and s >= 2:
                    # mbuf slot s%2 free once PE consumed sw s-2's tiles
                    gp.wait_ge(pe_c, pe_after_tiles_of_sw[s - 2])
                # SWDGE descriptor carveout caps a single gather call:
                # 1024 descriptors OK on HW, 2304 wedges the device.
                for c0 in range(0, ntc, GCHUNK):
                    nt = min(GCHUNK, ntc - c0)
                    tt = t0 + c0
                    n_idx = nt * TILE
                    off = int(tile_off[tt])
                    gp.dma_gather(
                        mbuf[:, s % 2, off:off + nt, :],
                        hn_d[hf * Ps[s]:N if hf else Ps[s], :],
                        sb_gidx[:, tt * 8:(tt + nt) * 8],
                        n_idx,
                        n_idx,
                        F,
                        queue_num=qrr % NQ,
                    ).then_inc(gsem[s % 2][hf][qrr % NQ], 16)
                    qrr += 1

        @block.tensor
        def _(pe):
            waited_ld = False
            for s in range(n_sw + 1):
                if s < n_sw:
                    for t in sw_tiles(s):
                        w = int(tile_window[t])
                        if first_tile[w] == t and w >= 4:
                            pe.wait_ge(aggc, w - 3)
                        if t in chunk_target:
                            # this chunk's gather call drained
                            sl, hf, q, tgt = chunk_target[t]
                            pe.wait_ge(gsem[sl][hf][q], tgt)
                        pe.wait_ge(sready, t + 1)
                        j = int(tile_off[t])
                        pe.matmul(
                            ps_agg[:, w % 4, 0:WIN],
                            mbuf[:, s % 2, j, :],
                            sbuf_S[:, s % 2, j, :],
                            start=(first_tile[w] == t),
                            stop=(last_tile[w] == t),
                        ).then_inc(pe_c)
                if s >= 1:
                    if not waited_ld:
                        pe.wait_ge(ld_rest, 16 * 4)  # nd/ones/bias/W loaded
                        waited_ld = True
                    for w in sw_windows[s - 1]:
                        bk = 2 * (w % 2)
                        # rank-1 bias init of the two output PSUM banks
                        if w >= 2:
                            pe.wait_ge(dvsc, w - 1)
                        pe.matmul(
                            ps_out[0:64, bk, 0:F],
                            sb_ones[:, 0:64],
                            sb_bias[:, :],
                            start=True,
                            stop=False,
                        ).then_inc(pe_c)
                        pe.matmul(
                            ps_out[0:64, bk + 1, 0:F],
                            sb_ones[:, 0:64],
                            sb_bias[:, :],
                            start=True,
                            stop=False,
                        ).then_inc(pe_c)
                        pe.wait_ge(aggc, w + 1)
                        # split W-matmul: even dst rows -> bank bk, odd -> bk+1
                        pe.matmul(
                            ps_out[0:64, bk, 0:F],
                            aggTs[:, w % 2, 0:WIN:2],
                            sb_W[:, :],
                            start=False,
                            stop=True,
                        ).then_inc(pe_c)
                        pe.matmul(
                            ps_out[0:64, bk + 1, 0:F],
                            aggTs[:, w % 2, 1:WIN:2],
                            sb_W[:, :],
                            start=False,
                            stop=True,
                        ).then_inc(pe_c)

        @block.scalar
        def _(act):
            act.wait_ge(ld_rest, 16 * 4)  # scale-copy needs sb_nd
            for s in range(1, n_sw + 1):
                for w in sw_windows[s - 1]:
                    tgt = pe_count_after[("tile", last_tile[w])]
                    if w >= 2:
                        tgt = max(tgt, pe_count_after[("wmm", w - 2)])
                    act.wait_ge(pe_c, tgt)
                    act.activation(
                        aggTs[:, w % 2, :],
                        ps_agg[:, w % 4, 0:WIN],
                        mybir.ActivationFunctionType.Copy,
                    ).then_inc(aggc)
                for w in sw_windows[s - 1]:
                    # outsb = ps_out * norm_dst (bias already in PSUM);
                    # partition p packs dst rows (2p, 2p+1) side by side
                    bk = 2 * (w % 2)
                    act.wait_ge(pe_c, pe_count_after[("wmm", w)])
                    if w >= 2:
                        act.wait_ge(osem[w % 2], 16 * (w // 2))
                    act.activation(
                        outsb[:, w % 2, 0:F],
                        ps_out[0:64, bk, 0:F],
                        mybir.ActivationFunctionType.Copy,
                        scale=sb_nd[:, 0, w:w + 1],
                    )
                    act.activation(
                        outsb[:, w % 2, F:2 * F],
                        ps_out[0:64, bk + 1, 0:F],
                        mybir.ActivationFunctionType.Copy,
                        scale=sb_nd[:, 1, w:w + 1],
                    ).then_inc(dvsc)

        @block.vector
        def _(dve):
            dve.wait_ge(ld_dve, 16 * 2)  # S-build needs sb_mdst + sb_iota
            for s in range(n_sw):
                if s >= 2:
                    # sbuf_S slot s%2 free once PE consumed sw s-2's tiles
                    dve.wait_ge(pe_c, pe_after_tiles_of_sw[s - 2])
                for t in sw_tiles(s):
                    j = int(tile_off[t])
                    dve.tensor_scalar(
                        sbuf_S[:, s % 2, j, :],
                        sb_iota[:, :],
                        sb_mdst[:, t:t + 1],
                        None,
                        AluOpType.is_equal,
                    ).then_inc(sready)

    nc.compile()
    return nc


# ------------------------------- entry point --------------------------------

def _run_spmd(nc, cores):
    """Execute the compiled program on 8 cores via PJRT/shard_map.

    Inputs are device_put with an explicit core sharding BEFORE the execute
    (run_bass_via_pjrt's host-numpy-per-call staging has crashed the exec
    unit on this program; the device-resident path is the one validated by
    test.py)."""
    import jax
    from jax.sharding import Mesh, NamedSharding, PartitionSpec
    from jax.experimental.shard_map import shard_map
    from concourse.bass2jax import (_bass_exec_p, install_neuronx_cc_hook,
                                    partition_id_tensor)

    install_neuronx_cc_hook()
    partition_name = nc.partition_id_tensor.name if nc.partition_id_tensor else None
    in_names, out_names, out_avals = [], [], []
    for alloc in nc.m.functions[0].allocations:
        if not isinstance(alloc, mybir.MemoryLocationSet):
            continue
        name = alloc.memorylocations[0].name
        if alloc.kind == "ExternalInput":
            if name != partition_name:
                in_names.append(name)
        elif alloc.kind == "ExternalOutput":
            out_names.append(name)
            out_avals.append(jax.core.ShapedArray(tuple(alloc.tensor_shape),
                                                  mybir.dt.np(alloc.dtype)))
    n_params = len(in_names)
    all_in_names = in_names + out_names + ([partition_name] if partition_name else [])

    def _body(*args):
        operands = list(args)
        if partition_name is not None:
            operands.append(partition_id_tensor())
        return tuple(_bass_exec_p.bind(
            *operands, out_avals=tuple(out_avals), in_names=tuple(all_in_names),
            out_names=tuple(out_names), lowering_input_output_aliases=(),
            sim_require_finite=True, sim_require_nnan=True, nc=nc))

    devices = jax.devices()[:C]
    mesh = Mesh(np.asarray(devices), ("core",))
    n_outs = len(out_names)
    sharded = jax.jit(shard_map(_body, mesh=mesh,
                                in_specs=(PartitionSpec("core"),) * (n_params + n_outs),
                                out_specs=(PartitionSpec("core"),) * n_outs,
                                check_rep=False),
                      donate_argnums=tuple(range(n_params, n_params + n_outs)),
                      keep_unused=True)

    sharding = NamedSharding(mesh, PartitionSpec("core"))
    concat_in = [np.concatenate([np.asarray(cores[c][nm]) for c in range(C)], axis=0)
                 for nm in in_names]
    dev_in = [jax.device_put(a, sharding) for a in concat_in]
    zeros = [jax.device_put(np.zeros((C * a.shape[0], *a.shape[1:]), a.dtype), sharding)
             for a in out_avals]
    out = sharded(*dev_in, *zeros)
    jax.block_until_ready(out)
    full = np.asarray(out[0]).reshape(C, *out_avals[0].shape)
    return [full[c] for c in range(C)]


def kernel(h, norm, W, b, src, dst):
    cores, pat = host_prep(h, norm, W, b, src, dst)
    nc = build_program(pat)

    outs_raw = _run_spmd(nc, cores)
    # out is [NW*WIN//2, 2F] bf16 with dst rows (2p, 2p+1) packed per row
    outs = [np.asarray(o).astype(np.float32).reshape(-1, F)[:R] for o in outs_raw]
    return np.ascontiguousarray(np.concatenate(outs, axis=0).astype(np.float32))

